# revision 1
# baseline (speedup 1.0000x reference)
"""Trainium2 Bass kernel for DeepSelfAttention (N=8192, D=1024) on 8 NeuronCores.

Strategy (row-parallel attention):
  - Shard the N=8192 rows of x across 8 cores (1024 rows each); replicate weights.
  - Each core computes Q/K/V projections for its row shard in feature-major
    layout (contraction dim on SBUF partitions); all operand transposes are
    done on the TensorEngine (fp32 transpose + fused fp16 cast on the
    PSUM->SBUF copy).
  - K^T and V shards are AllGathered across the 8 cores in two fp16 chunks
    (k-halves), concatenated per chunk into one flat collective, so attention
    on chunk 0 overlaps the second AllGather; Q projection and the MLP weight
    transposes fill the first AllGather's latency.
  - Flash-style one-pass attention: scores^T tiles [k=128, q=512] accumulate
    over feature tiles in PSUM, exp on ScalarE (scale=1/32 fused; scores for
    this model are provably in [-3, 3] so no max-subtraction is needed),
    A@V accumulated per (chunk, block) in PSUM with one bank-group at a time
    (PSUM start=True clears has_written for the whole bank) and flushed to an
    SBUF fp32 accumulator; softmax denominator via a ones-vector matmul.
  - The V bias is folded into the post-softmax normalize (softmax rows sum
    to 1), where it is a per-partition bias.
  - 3-layer MLP + final projection, feature-major.
All matmul operands are fp16 (full PE rate on TRN2) with fp32 PSUM
accumulation; end-to-end max rel err vs the fp32 reference is ~4e-4.
"""

import os

import numpy as np

import concourse.mybir as mybir
import concourse.tile as tile
from concourse import bacc
from concourse import bass_utils
from concourse.masks import make_identity

P = 128
D = 1024
N = 8192
NCORES = 8
NS = N // NCORES          # 1024 rows per core
DT = D // P               # 8 feature tiles
QG = 4                    # attention q groups per core
QGS = NS // QG            # 256
KB = 8                    # k blocks (one per source core)
KTB = NS // P             # 8 k tiles per block
KTH = KTB // 2            # 4 k tiles per chunk-block
CH = NS // 2              # 512 keys per chunk
KSZ = D * CH              # K-chunk elements in the flat collective buffer
VSZ = CH * D
F16 = mybir.dt.float16
F32 = mybir.dt.float32
AF = mybir.ActivationFunctionType
ALU = mybir.AluOpType

SCALE = 1.0 / np.sqrt(np.float32(D)).astype(np.float32)  # 0.03125

_CACHE = {}


def _transpose_pe(nc, raw_pool, ptr_pool, ident, src_ap, dst_tile):
    """src_ap: DRAM fp32 [R, C] -> dst_tile: SBUF fp16 [P, C//P, R] = src.T,
    via TensorEngine transpose (fp32) + ScalarE PSUM->SBUF copy w/ fp16 cast."""
    R, C = src_ap.shape
    for i in range(R // P):
        r = raw_pool.tile([P, C], F32, tag="raw")
        nc.sync.dma_start(r[:], src_ap[i * P:(i + 1) * P, :])
        for j in range(C // P):
            pst = ptr_pool.tile([P, P], F32, tag="ptr")
            nc.tensor.transpose(pst[:], r[:, j * P:(j + 1) * P], ident[:])
            nc.vector.tensor_copy(dst_tile[:, j, i * P:(i + 1) * P], pst[:])


def _build():
    nc = bacc.Bacc("TRN2", target_bir_lowering=False, debug=False,
                   num_devices=NCORES)
    xs = nc.dram_tensor("xs", [NS, D], F32, kind="ExternalInput").ap()
    W = {}
    for w in ("wq", "wk", "wv", "w1", "w2", "w3"):
        W[w] = nc.dram_tensor(w, [D, D], F32, kind="ExternalInput").ap()
    B = {}
    for b in ("bq", "bk", "bv", "b1", "b2", "b3"):
        B[b] = nc.dram_tensor(b, [D], F32, kind="ExternalInput").ap()
    fw = nc.dram_tensor("fw", [D], F32, kind="ExternalInput").ap()
    out = nc.dram_tensor("out", [1, NS], F32, kind="ExternalOutput").ap()
    debug = bool(os.environ.get("K_DEBUG"))
    dbg = {}
    if debug:
        for nm, shp, dt_ in (("dq", [D, NS], F16), ("drs", [1, NS], F32),
                             ("datt", [D, NS], F16), ("dy1", [D, NS], F16)):
            dbg[nm] = nc.dram_tensor(nm, shp, dt_, kind="ExternalOutput").ap()

    with tile.TileContext(nc) as tc:
        with (
            tc.tile_pool(name="persist", bufs=1) as pers,
            tc.tile_pool(name="dram", bufs=1, space="DRAM") as dram,
        ):
            # ---- persistent SBUF tiles ----
            qt = pers.tile([P, DT, NS], F16, tag="qt")          # Q^T
            wT = {w: pers.tile([P, DT, D], F16, tag=f"{w}T", name=f"{w}T")
                  for w in ("w1", "w2", "w3")}
            bsb = {b: pers.tile([P, DT], F32, tag=f"{b}sb", name=f"{b}sb")
                   for b in B}
            fwh = pers.tile([P, DT], F16, tag="fwh")
            ones_h = pers.tile([P, 1], F16, tag="ones")
            ones_row = pers.tile([1, P], F32, tag="ones_row")
            ident = pers.tile([P, P], F32, tag="ident")
            rs = pers.tile([1, NS], F32, tag="rs")              # softmax denom

            # ---- DRAM scratch: flat (K-chunk | V-chunk) collective buffers
            kv_d = [dram.tile([KSZ + VSZ], F16, name=f"kv_d{c}")
                    for c in range(2)]
            kvag = [dram.tile([NCORES * (KSZ + VSZ)], F16, name=f"kvag{c}",
                              addr_space="Shared")
                    for c in range(2)]

            # ---- constants ----
            for b in B:
                nc.sync.dma_start(bsb[b][:], B[b].rearrange("(t p) -> p t", p=P))
            fwf = pers.tile([P, DT], F32, tag="fwf")
            nc.sync.dma_start(fwf[:], fw.rearrange("(t p) -> p t", p=P))
            nc.vector.tensor_copy(fwh[:], fwf[:])
            nc.gpsimd.memset(ones_h[:], 1.0)
            nc.gpsimd.memset(ones_row[:], 1.0)
            make_identity(nc, ident[:])

            # ---- early pool: dies after projections ----
            early = tc.alloc_tile_pool(name="early", bufs=1)
            xsT = early.tile([P, DT, NS], F16, tag="xsT")
            for w in ("wq", "wk", "wv"):
                wT[w] = early.tile([P, DT, D], F16, tag=f"{w}T", name=f"{w}T")
            kts = early.tile([P, DT, NS], F16, tag="kts")       # K^T shard
            vs = early.tile([P, KTB, D], F16, tag="vs")         # V shard

            with (
                tc.tile_pool(name="raw", bufs=3) as raw,
                tc.tile_pool(name="ppj", bufs=4, space="PSUM") as ppj,
            ):
                # transposes on PE: x, then K/V weights (gate the AllGather),
                # then Q's
                _transpose_pe(nc, raw, ppj, ident, xs, xsT)
                for w in ("wk", "wv"):
                    _transpose_pe(nc, raw, ppj, ident, W[w], wT[w])

                # K^T = Wk @ xs^T + bk; emit + ship per k-half
                for h in range(2):
                    for dt in range(DT):
                        ps = ppj.tile([P, 512], F32, tag="ppj")
                        for et in range(DT):
                            nc.tensor.matmul(
                                ps[:],
                                wT["wk"][:, et, dt * P:(dt + 1) * P],
                                xsT[:, et, h * 512:(h + 1) * 512],
                                start=(et == 0), stop=(et == DT - 1))
                        nc.vector.tensor_tensor(
                            kts[:, dt, h * 512:(h + 1) * 512], ps[:],
                            bsb["bk"][:, dt:dt + 1].to_broadcast([P, 512]),
                            ALU.add)
                    nc.sync.dma_start(
                        kv_d[h][0:KSZ].rearrange("(t p k) -> p t k", p=P, k=CH),
                        kts[:, :, h * CH:(h + 1) * CH])
                # V = xs @ Wv.T (bias folded into post-softmax normalize)
                for h in range(2):
                    for kt in range(h * KTH, (h + 1) * KTH):
                        for dh in range(2):
                            ps = ppj.tile([P, 512], F32, tag="ppj")
                            for et in range(DT):
                                nc.tensor.matmul(
                                    ps[:],
                                    xsT[:, et, kt * P:(kt + 1) * P],
                                    wT["wv"][:, et, dh * 512:(dh + 1) * 512],
                                    start=(et == 0), stop=(et == DT - 1))
                            nc.vector.tensor_copy(
                                vs[:, kt, dh * 512:(dh + 1) * 512], ps[:])
                    nc.sync.dma_start(
                        kv_d[h][KSZ:].rearrange("(t p d) -> p t d", p=P, d=D),
                        vs[:, h * KTH:(h + 1) * KTH, :])
                    nc.gpsimd.collective_compute(
                        "AllGather", ALU.bypass,
                        replica_groups=[list(range(NCORES))],
                        ins=[kv_d[h].opt()], outs=[kvag[h].opt()])

                # work that fills the first AllGather's latency:
                # Q^T projection + MLP weight transposes
                _transpose_pe(nc, raw, ppj, ident, W["wq"], wT["wq"])
                for dt in range(DT):
                    for h in range(2):
                        ps = ppj.tile([P, 512], F32, tag="ppj")
                        for et in range(DT):
                            nc.tensor.matmul(
                                ps[:],
                                wT["wq"][:, et, dt * P:(dt + 1) * P],
                                xsT[:, et, h * 512:(h + 1) * 512],
                                start=(et == 0), stop=(et == DT - 1))
                        nc.vector.tensor_tensor(
                            qt[:, dt, h * 512:(h + 1) * 512], ps[:],
                            bsb["bq"][:, dt:dt + 1].to_broadcast([P, 512]),
                            ALU.add)
                for w in ("w1", "w2", "w3"):
                    _transpose_pe(nc, raw, ppj, ident, W[w], wT[w])

            early.release()

            if debug:
                nc.sync.dma_start(dbg["dq"].rearrange("(t p) k -> p t k", p=P),
                                  qt[:])

            # ---- attention over 2 chunks x 8 blocks ----
            pacc = tc.alloc_tile_pool(name="pacc", bufs=1)
            attacc = pacc.tile([P, DT, NS], F32, tag="attacc")
            with (
                tc.tile_pool(name="kv", bufs=3) as kv,
                tc.tile_pool(name="ex", bufs=8) as exp_pool,
                tc.tile_pool(name="psc", bufs=2, space="PSUM") as psc,
                tc.tile_pool(name="pat", bufs=4, space="PSUM") as pat,
                tc.tile_pool(name="prs", bufs=2, space="PSUM") as prs,
            ):
                for ch in range(2):
                    base = kvag[ch]
                    for kb in range(KB):
                        off = kb * (KSZ + VSZ)
                        ktb = kv.tile([P, DT, CH], F16, tag="ktb")
                        vb = kv.tile([P, KTH, D], F16, tag="vb")
                        nc.sync.dma_start(
                            ktb[:],
                            base[off:off + KSZ].rearrange(
                                "(t p k) -> p t k", p=P, k=CH))
                        nc.sync.dma_start(
                            vb[:],
                            base[off + KSZ:off + KSZ + VSZ].rearrange(
                                "(t p d) -> p t d", p=P, d=D))
                        first_blk = ch == 0 and kb == 0
                        for qp in range(2):
                            qpsl = slice(qp * 512, (qp + 1) * 512)
                            rs_ps = prs.tile([1, 512], F32, tag="prs")
                            exs = []
                            for kt in range(KTH):
                                sc = psc.tile([P, 512], F32, tag="psc")
                                for dt in range(DT):
                                    nc.tensor.matmul(
                                        sc[:],
                                        ktb[:, dt, kt * P:(kt + 1) * P],
                                        qt[:, dt, qpsl],
                                        start=(dt == 0), stop=(dt == DT - 1))
                                ex = exp_pool.tile([P, 512], F16, tag="ex",
                                                   name=f"ex{kt}")
                                nc.scalar.activation(ex[:], sc[:], AF.Exp,
                                                     scale=float(SCALE))
                                nc.tensor.matmul(rs_ps[:], ones_h[:], ex[:],
                                                 start=(kt == 0),
                                                 stop=(kt == KTH - 1),
                                                 skip_group_check=True)
                                exs.append(ex)
                            if first_blk:
                                nc.vector.tensor_copy(rs[0:1, qpsl], rs_ps[:])
                            else:
                                nc.vector.tensor_tensor(
                                    rs[0:1, qpsl], rs_ps[:], rs[0:1, qpsl],
                                    ALU.add)
                            # A@V, one PSUM-bank accumulation group at a time
                            # (start=True clears has_written bank-wide)
                            for hq in range(2):
                                qsl = slice(qp * 512 + hq * QGS,
                                            qp * 512 + (hq + 1) * QGS)
                                att_ps = [pat.tile([P, 2, QGS], F32, tag="pat",
                                                   name=f"att_ps{_j}")
                                          for _j in range(4)]
                                for dt in range(DT):
                                    for kt in range(KTH):
                                        nc.tensor.matmul(
                                            att_ps[dt // 2][:, dt % 2, :],
                                            vb[:, kt, dt * P:(dt + 1) * P],
                                            exs[kt][:, hq * QGS:(hq + 1) * QGS],
                                            start=(kt == 0),
                                            stop=(kt == KTH - 1),
                                            skip_group_check=True)
                                for j in range(4):
                                    dsl = (slice(None), slice(2 * j, 2 * j + 2),
                                           qsl)
                                    if first_blk:
                                        nc.vector.tensor_copy(attacc[dsl],
                                                              att_ps[j][:])
                                    else:
                                        nc.vector.tensor_tensor(
                                            attacc[dsl], att_ps[j][:],
                                            attacc[dsl], ALU.add)

            # ---- normalize + MLP + final ----
            with (
                tc.tile_pool(name="acts", bufs=2) as acts,
                tc.tile_pool(name="pml", bufs=4, space="PSUM") as pml,
            ):
                recip = acts.tile([1, NS], F32, tag="recip")
                out_sb = acts.tile([1, NS], F32, tag="out_sb")
                nc.vector.reciprocal(recip[:], rs[:])
                attn_h = acts.tile([P, DT, NS], F16, tag="y")
                for h in range(2):
                    qsl = slice(h * 512, (h + 1) * 512)
                    rb = pml.tile([P, 512], F32, tag="pml")
                    nc.tensor.matmul(rb[:], ones_row[:], recip[0:1, qsl])
                    for dt in range(DT):
                        nc.vector.tensor_tensor(
                            attn_h[:, dt, qsl], attacc[:, dt, qsl], rb[:],
                            ALU.mult)
                        nc.vector.tensor_tensor(
                            attn_h[:, dt, qsl], attn_h[:, dt, qsl],
                            bsb["bv"][:, dt:dt + 1].to_broadcast([P, 512]),
                            ALU.add)
                if debug:
                    nc.sync.dma_start(dbg["drs"][:], rs[:])
                    nc.sync.dma_start(
                        dbg["datt"].rearrange("(t p) q -> p t q", p=P),
                        attn_h[:])
                cur = attn_h
                for wname, bname in (("w1", "b1"), ("w2", "b2"), ("w3", "b3")):
                    nxt = acts.tile([P, DT, NS], F16, tag="y")
                    for ft in range(DT):
                        for h in range(2):
                            ps = pml.tile([P, 512], F32, tag="pml")
                            for dt in range(DT):
                                nc.tensor.matmul(
                                    ps[:],
                                    wT[wname][:, dt, ft * P:(ft + 1) * P],
                                    cur[:, dt, h * 512:(h + 1) * 512],
                                    start=(dt == 0), stop=(dt == DT - 1))
                            nc.scalar.activation(
                                nxt[:, ft, h * 512:(h + 1) * 512], ps[:],
                                AF.Relu, bias=bsb[bname][:, ft:ft + 1])
                    if debug and wname == "w1":
                        nc.sync.dma_start(
                            dbg["dy1"].rearrange("(t p) q -> p t q", p=P),
                            nxt[:])
                    cur = nxt
                for h in range(2):
                    ps = pml.tile([1, 512], F32, tag="pfin")
                    for ft in range(DT):
                        nc.tensor.matmul(
                            ps[:], fwh[:, ft:ft + 1],
                            cur[:, ft, h * 512:(h + 1) * 512],
                            start=(ft == 0), stop=(ft == DT - 1))
                    nc.vector.tensor_copy(out_sb[0:1, h * 512:(h + 1) * 512],
                                          ps[:])
                nc.sync.dma_start(out[:], out_sb[:])
            pacc.release()

    nc.compile()
    return nc


def _get_nc():
    if "nc" not in _CACHE:
        _CACHE["nc"] = _build()
    return _CACHE["nc"]


def kernel(**inputs):
    nc = _get_nc()
    x = np.ascontiguousarray(np.asarray(inputs["x"], dtype=np.float32))
    names = {"wq": "Wq", "wk": "Wk", "wv": "Wv", "w1": "W1", "w2": "W2",
             "w3": "W3", "bq": "bq", "bk": "bk", "bv": "bv", "b1": "b1",
             "b2": "b2", "b3": "b3"}
    shared = {k: np.ascontiguousarray(np.asarray(inputs[v], dtype=np.float32))
              for k, v in names.items()}
    shared["fw"] = np.ascontiguousarray(
        np.asarray(inputs["final_weight"], dtype=np.float32).reshape(D))
    in_maps = []
    for c in range(NCORES):
        m = dict(shared)
        m["xs"] = np.ascontiguousarray(x[c * NS:(c + 1) * NS, :])
        in_maps.append(m)
    res = bass_utils.run_bass_kernel_spmd(
        nc, in_maps, core_ids=list(range(NCORES)))
    if os.environ.get("K_DEBUG"):
        kernel.debug_results = res.results
    return np.concatenate(
        [res.results[c]["out"].reshape(NS) for c in range(NCORES)])



# revision 2
# speedup vs baseline: 1.2175x; 1.2175x over previous
"""Trainium2 Bass kernel for DeepSelfAttention (N=8192, D=1024) on 8 NeuronCores.

Strategy (row-parallel attention):
  - Shard the N=8192 rows of x across 8 cores (1024 rows each); replicate
    weights.  All weight matrices and the x shards are pre-transposed and cast
    to fp16 on the HOST, so the device does zero layout work: every matmul
    operand is DMA'd directly into its contraction-major SBUF layout.
  - Bias algebra (host-folded):
      * bk drops out of softmax entirely (it shifts every score of a row by a
        q-dependent constant).
      * bv is folded into the first MLP bias: b1' = b1 + W1 @ bv (softmax rows
        sum to 1).
    Only bq survives on device (added to Q after the projection).
  - Each core computes K^T and V for its row shard, ships them as two fp16
    (K-half | V-half) chunks, AllGathered across the 8 cores; the Q projection
    and the first attention blocks cover the AllGather latency.
  - Flash-style one-pass attention: scores^T tiles [k=128, q=512] accumulate
    over feature tiles in PSUM, exp on ScalarE (scale=1/32 fused; no
    max-subtraction needed, scores are small), softmax denominator via a
    ones-vector matmul, A@V accumulated per (block, dt-half) in PSUM with
    full 512-wide moving operands and flushed to an SBUF fp32 accumulator.
  - Normalize via PE broadcast of 1/rowsum + fused-cast multiply, then the
    3-layer MLP + final projection, feature-major, pipelined by column half.
All matmul operands are fp16 (full PE rate on TRN2) with fp32 PSUM
accumulation; end-to-end max rel err vs the fp32 reference is ~4e-4.
"""

import numpy as np

import concourse.mybir as mybir
import concourse.tile as tile
from concourse import bacc
from concourse import bass_utils

P = 128
D = 1024
N = 8192
NCORES = 8
NS = N // NCORES          # 1024 rows per core
DT = D // P               # 8 feature tiles
KB = 8                    # k blocks (one per source core)
KTB = NS // P             # 8 k tiles per block
KTH = KTB // 2            # 4 k tiles per chunk-block
CH = NS // 2              # 512 keys per chunk
KSZ = D * CH              # K-chunk elements in the flat collective buffer
VSZ = CH * D
F16 = mybir.dt.float16
F32 = mybir.dt.float32
AF = mybir.ActivationFunctionType
ALU = mybir.AluOpType

SCALE = 1.0 / np.sqrt(np.float32(D)).astype(np.float32)  # 0.03125

_CACHE = {}


def _build():
    nc = bacc.Bacc("TRN2", target_bir_lowering=False, debug=False,
                   num_devices=NCORES)
    xsT = nc.dram_tensor("xsT", [D, NS], F16, kind="ExternalInput").ap()
    W = {}
    for w in ("wqT", "wkT", "wvT", "w1T", "w2T", "w3T"):
        W[w] = nc.dram_tensor(w, [D, D], F16, kind="ExternalInput").ap()
    B = {}
    for b in ("bq", "b1", "b2", "b3"):
        B[b] = nc.dram_tensor(b, [D], F32, kind="ExternalInput").ap()
    fw = nc.dram_tensor("fw", [D], F16, kind="ExternalInput").ap()
    out = nc.dram_tensor("out", [1, NS], F32, kind="ExternalOutput").ap()

    with tile.TileContext(nc) as tc:
        with (
            tc.tile_pool(name="persist", bufs=1) as pers,
            tc.tile_pool(name="dram", bufs=1, space="DRAM") as dram,
        ):
            # ---- persistent SBUF tiles ----
            qt = pers.tile([P, DT, NS], F16, tag="qt")          # Q^T
            wT = {w: pers.tile([P, DT, D], F16, tag=f"{w}", name=f"{w}")
                  for w in ("w1T", "w2T", "w3T")}
            bsb = {b: pers.tile([P, DT], F32, tag=f"{b}sb", name=f"{b}sb")
                   for b in B}
            fwh = pers.tile([P, DT], F16, tag="fwh")
            ones_h = pers.tile([P, 1], F16, tag="ones")
            ones_row = pers.tile([1, P], F16, tag="ones_row")
            rs = pers.tile([1, NS], F32, tag="rs")              # softmax denom
            rs_h = pers.tile([1, NS], F16, tag="rs_h")
            kts = pers.tile([P, DT, NS], F16, tag="kts")        # K^T shard
            vs = pers.tile([P, KTB, D], F16, tag="vs")          # V shard

            # ---- DRAM scratch: flat (K-chunk | V-chunk) collective buffers
            kv_d = [dram.tile([KSZ + VSZ], F16, name=f"kv_d{c}")
                    for c in range(2)]
            kvag = [dram.tile([NCORES * (KSZ + VSZ)], F16, name=f"kvag{c}",
                              addr_space="Shared")
                    for c in range(2)]

            # ---- constants ----
            for b in B:
                nc.sync.dma_start(bsb[b][:], B[b].rearrange("(t p) -> p t", p=P))
            nc.sync.dma_start(fwh[:], fw.rearrange("(t p) -> p t", p=P))
            nc.gpsimd.memset(ones_h[:], 1.0)
            nc.gpsimd.memset(ones_row[:], 1.0)

            # ---- early pool: dies after projections ----
            early = tc.alloc_tile_pool(name="early", bufs=1)
            xsb = early.tile([P, DT, NS], F16, tag="xsb")
            wesb = {w: early.tile([P, DT, D], F16, tag=f"{w}", name=f"{w}")
                    for w in ("wqT", "wkT", "wvT")}
            nc.sync.dma_start(xsb[:], xsT.rearrange("(t p) n -> p t n", p=P))
            for w in ("wkT", "wvT", "wqT"):
                nc.sync.dma_start(wesb[w][:],
                                  W[w].rearrange("(t p) m -> p t m", p=P))

            with tc.tile_pool(name="ppj", bufs=4, space="PSUM") as ppj:
                # K^T = Wk @ xs^T (no bias: bk cancels in softmax);
                # V = xs @ Wv.T (bias folded into b1'); ship + AllGather
                # one (K-half | V-half) chunk at a time.
                for h in range(2):
                    for dt in range(DT):
                        ps = ppj.tile([P, 512], F32, tag="ppj")
                        for et in range(DT):
                            nc.tensor.matmul(
                                ps[:],
                                wesb["wkT"][:, et, dt * P:(dt + 1) * P],
                                xsb[:, et, h * 512:(h + 1) * 512],
                                start=(et == 0), stop=(et == DT - 1))
                        nc.scalar.activation(
                            kts[:, dt, h * 512:(h + 1) * 512], ps[:], AF.Copy)
                    nc.sync.dma_start(
                        kv_d[h][0:KSZ].rearrange("(t p k) -> p t k", p=P, k=CH),
                        kts[:, :, h * CH:(h + 1) * CH])
                    for kt in range(h * KTH, (h + 1) * KTH):
                        for dh in range(2):
                            ps = ppj.tile([P, 512], F32, tag="ppj")
                            for et in range(DT):
                                nc.tensor.matmul(
                                    ps[:],
                                    xsb[:, et, kt * P:(kt + 1) * P],
                                    wesb["wvT"][:, et, dh * 512:(dh + 1) * 512],
                                    start=(et == 0), stop=(et == DT - 1))
                            nc.scalar.activation(
                                vs[:, kt, dh * 512:(dh + 1) * 512], ps[:],
                                AF.Copy)
                    nc.sync.dma_start(
                        kv_d[h][KSZ:].rearrange("(t p d) -> p t d", p=P, d=D),
                        vs[:, h * KTH:(h + 1) * KTH, :])
                    nc.gpsimd.collective_compute(
                        "AllGather", ALU.bypass,
                        replica_groups=[list(range(NCORES))],
                        ins=[kv_d[h].opt()], outs=[kvag[h].opt()])

                # Q^T projection fills the first AllGather's latency
                for dt in range(DT):
                    for h in range(2):
                        ps = ppj.tile([P, 512], F32, tag="ppj")
                        for et in range(DT):
                            nc.tensor.matmul(
                                ps[:],
                                wesb["wqT"][:, et, dt * P:(dt + 1) * P],
                                xsb[:, et, h * 512:(h + 1) * 512],
                                start=(et == 0), stop=(et == DT - 1))
                        nc.vector.tensor_tensor(
                            qt[:, dt, h * 512:(h + 1) * 512], ps[:],
                            bsb["bq"][:, dt:dt + 1].to_broadcast([P, 512]),
                            ALU.add)
            early.release()

            # DMA in the MLP weights while attention runs
            for w in ("w1T", "w2T", "w3T"):
                nc.sync.dma_start(wT[w][:],
                                  W[w].rearrange("(t p) m -> p t m", p=P))

            # ---- attention over 2 chunks x 8 blocks ----
            pacc = tc.alloc_tile_pool(name="pacc", bufs=1)
            attacc = pacc.tile([P, DT, NS], F32, tag="attacc")
            with (
                tc.tile_pool(name="kv", bufs=2) as kv,
                tc.tile_pool(name="ex", bufs=8) as exp_pool,
                tc.tile_pool(name="psc", bufs=2, space="PSUM") as psc,
                tc.tile_pool(name="pat", bufs=5, space="PSUM") as pat,
                tc.tile_pool(name="prs", bufs=1, space="PSUM") as prs,
            ):
                for ch in range(2):
                    base = kvag[ch]
                    for kb in range(KB):
                        off = kb * (KSZ + VSZ)
                        ktb = kv.tile([P, DT, CH], F16, tag="ktb")
                        vb = kv.tile([P, KTH, D], F16, tag="vb")
                        nc.sync.dma_start(
                            ktb[:],
                            base[off:off + KSZ].rearrange(
                                "(t p k) -> p t k", p=P, k=CH))
                        nc.sync.dma_start(
                            vb[:],
                            base[off + KSZ:off + KSZ + VSZ].rearrange(
                                "(t p d) -> p t d", p=P, d=D))
                        first_blk = ch == 0 and kb == 0
                        for qp in range(2):
                            qpsl = slice(qp * 512, (qp + 1) * 512)
                            rs_ps = prs.tile([1, 512], F32, tag="prs")
                            exs = []
                            for kt in range(KTH):
                                sc = psc.tile([P, 512], F32, tag="psc")
                                for dt in range(DT):
                                    nc.tensor.matmul(
                                        sc[:],
                                        ktb[:, dt, kt * P:(kt + 1) * P],
                                        qt[:, dt, qpsl],
                                        start=(dt == 0), stop=(dt == DT - 1))
                                ex = exp_pool.tile([P, 512], F16, tag="ex",
                                                   name=f"ex{kt}")
                                nc.scalar.activation(ex[:], sc[:], AF.Exp,
                                                     scale=float(SCALE))
                                nc.tensor.matmul(rs_ps[:], ones_h[:], ex[:],
                                                 start=(kt == 0),
                                                 stop=(kt == KTH - 1),
                                                 skip_group_check=True)
                                exs.append(ex)
                            if first_blk:
                                nc.vector.tensor_copy(rs[0:1, qpsl], rs_ps[:])
                            else:
                                nc.vector.tensor_tensor(
                                    rs[0:1, qpsl], rs_ps[:], rs[0:1, qpsl],
                                    ALU.add)
                            # A@V: 512-wide moving operand; accumulate the
                            # 4 kt steps per PSUM bank; dt in two halves so
                            # 4 banks + a 5-deep pool rotation keep the PE
                            # from waiting on the previous group's flush.
                            for dh in range(2):
                                att_ps = [pat.tile([P, 512], F32, tag="pat",
                                                   name=f"att_ps{_j}")
                                          for _j in range(4)]
                                for j in range(4):
                                    dt = dh * 4 + j
                                    for kt in range(KTH):
                                        nc.tensor.matmul(
                                            att_ps[j][:],
                                            vb[:, kt, dt * P:(dt + 1) * P],
                                            exs[kt][:],
                                            start=(kt == 0),
                                            stop=(kt == KTH - 1),
                                            skip_group_check=True)
                                for j in range(4):
                                    dsl = (slice(None), dh * 4 + j, qpsl)
                                    if first_blk:
                                        nc.vector.tensor_copy(attacc[dsl],
                                                              att_ps[j][:])
                                    else:
                                        nc.vector.tensor_tensor(
                                            attacc[dsl], att_ps[j][:],
                                            attacc[dsl], ALU.add)

            # ---- normalize + MLP + final, pipelined by column half ----
            with (
                tc.tile_pool(name="acts", bufs=4) as acts,
                tc.tile_pool(name="pml", bufs=4, space="PSUM") as pml,
            ):
                nc.vector.tensor_copy(rs_h[:], rs[:])
                out_sb = acts.tile([1, NS], F32, tag="out_sb")
                recips = []
                for h in range(2):
                    qsl = slice(h * 512, (h + 1) * 512)
                    rb = pml.tile([P, 512], F32, tag="pml")
                    nc.tensor.matmul(rb[:], ones_row[:], rs_h[0:1, qsl])
                    recip = acts.tile([P, 512], F32, tag="recip",
                                      name=f"recip{h}")
                    scratch = acts.tile([P, 512], F32, tag="rscratch",
                                        name=f"rscratch{h}")
                    nc.vector.reciprocal_approx_accurate(recip[:], rb[:],
                                                         scratch[:])
                    recips.append(recip)
                for h in range(2):
                    qsl = slice(h * 512, (h + 1) * 512)
                    attn_h = acts.tile([P, DT, 512], F16, tag="y",
                                       name=f"attn_h{h}")
                    for dt in range(DT):
                        nc.vector.tensor_tensor(
                            attn_h[:, dt, :], attacc[:, dt, qsl],
                            recips[h][:], ALU.mult)
                    cur = attn_h
                    for wname, bname in (("w1T", "b1"), ("w2T", "b2"),
                                         ("w3T", "b3")):
                        nxt = acts.tile([P, DT, 512], F16, tag="y",
                                        name=f"{wname}y{h}")
                        for ft in range(DT):
                            ps = pml.tile([P, 512], F32, tag="pml")
                            for dt in range(DT):
                                nc.tensor.matmul(
                                    ps[:],
                                    wT[wname][:, dt, ft * P:(ft + 1) * P],
                                    cur[:, dt, :],
                                    start=(dt == 0), stop=(dt == DT - 1))
                            nc.scalar.activation(
                                nxt[:, ft, :], ps[:],
                                AF.Relu, bias=bsb[bname][:, ft:ft + 1])
                        cur = nxt
                    ps = pml.tile([1, 512], F32, tag="pfin")
                    for ft in range(DT):
                        nc.tensor.matmul(
                            ps[:], fwh[:, ft:ft + 1], cur[:, ft, :],
                            start=(ft == 0), stop=(ft == DT - 1))
                    nc.vector.tensor_copy(out_sb[0:1, qsl], ps[:])
                nc.sync.dma_start(out[:], out_sb[:])
            pacc.release()

    nc.compile()
    return nc


def _get_nc():
    if "nc" not in _CACHE:
        _CACHE["nc"] = _build()
    return _CACHE["nc"]


def make_in_maps(inputs):
    """Host-side sharding/layout: transpose + fp16-cast the weights and the
    x shards, fold bv into b1."""
    f32 = np.float32
    x = np.asarray(inputs["x"], dtype=f32)
    shared = {}
    for dev, ref in (("wqT", "Wq"), ("wkT", "Wk"), ("wvT", "Wv"),
                     ("w1T", "W1"), ("w2T", "W2"), ("w3T", "W3")):
        shared[dev] = np.ascontiguousarray(
            np.asarray(inputs[ref], dtype=f32).T.astype(np.float16))
    b1p = (np.asarray(inputs["b1"], dtype=f32)
           + np.asarray(inputs["W1"], dtype=f32)
           @ np.asarray(inputs["bv"], dtype=f32)).astype(f32)
    shared["bq"] = np.ascontiguousarray(np.asarray(inputs["bq"], dtype=f32))
    shared["b1"] = np.ascontiguousarray(b1p)
    shared["b2"] = np.ascontiguousarray(np.asarray(inputs["b2"], dtype=f32))
    shared["b3"] = np.ascontiguousarray(np.asarray(inputs["b3"], dtype=f32))
    shared["fw"] = np.ascontiguousarray(
        np.asarray(inputs["final_weight"], dtype=f32).reshape(D)
        .astype(np.float16))
    in_maps = []
    for c in range(NCORES):
        m = dict(shared)
        m["xsT"] = np.ascontiguousarray(
            x[c * NS:(c + 1) * NS, :].T.astype(np.float16))
        in_maps.append(m)
    return in_maps


def kernel(**inputs):
    nc = _get_nc()
    res = bass_utils.run_bass_kernel_spmd(
        nc, make_in_maps(inputs), core_ids=list(range(NCORES)))
    return np.concatenate(
        [res.results[c]["out"].reshape(NS) for c in range(NCORES)])


# revision 8
# speedup vs baseline: 1.2793x; 1.0508x over previous
"""Trainium2 Bass kernel for DeepSelfAttention (N=8192, D=1024) on 8 NeuronCores.

Strategy (row-parallel attention):
  - Shard the N=8192 rows of x across 8 cores (1024 rows each); replicate
    weights.  All weight matrices and the x shards are pre-transposed and cast
    to fp16 on the HOST, so the device does zero layout work: every matmul
    operand is DMA'd directly into its contraction-major SBUF layout.
  - Bias algebra (host-folded):
      * bk drops out of softmax entirely (it shifts every score of a row by a
        q-dependent constant).
      * bv is folded into the first MLP bias: b1' = b1 + W1 @ bv (softmax rows
        sum to 1).
    Only bq survives on device (added to Q after the projection).
  - Each core computes K^T and V for its row shard and ships them in three
    fp16 chunks of [1, 2, 5] key-tiles; each chunk is AllGathered as soon as
    it is ready, so the first chunk's collective fully overlaps the remaining
    projections and each later chunk overlaps attention on earlier chunks.
  - Attention processes 128-key units in groups of 8 (1024 keys): scores^T
    tiles [k=128, q=512] accumulate over feature tiles in PSUM, exp on ScalarE
    (scale=1/32 fused; no max-subtraction needed, scores are small), softmax
    denominator via a ones-vector matmul, A@V accumulated across the whole
    group (8-matmul PSUM chains) and flushed to an SBUF fp32 accumulator.
  - Normalize via PE broadcast of 1/rowsum (emitted inside the last attention
    group so the DVE work hides under the last A@V chains), then the 3-layer
    MLP + final projection, feature-major, pipelined by column half.
All matmul operands are fp16 with fp32 PSUM accumulation; end-to-end max rel
err vs the fp32 reference is ~5e-4.
"""

import numpy as np

import concourse.mybir as mybir
import concourse.tile as tile
from concourse import bacc
from concourse import bass_utils

P = 128
D = 1024
N = 8192
NCORES = 8
NS = N // NCORES          # 1024 rows per core
DT = D // P               # 8 feature tiles
KB = 8                    # k blocks (one per source core)
KTB = NS // P             # 8 k tiles per block
CHUNK_KTS = [[0], [1, 2], [3, 4, 5, 6, 7]]   # kt split per AllGather chunk
USZ = P * D               # elements per K or V unit (128 keys x 1024 feat)
G = 8                     # units per attention group (PSUM chain length)
F16 = mybir.dt.float16
F32 = mybir.dt.float32
AF = mybir.ActivationFunctionType
ALU = mybir.AluOpType

SCALE = 1.0 / np.sqrt(np.float32(D)).astype(np.float32)  # 0.03125

_CACHE = {}


def _build():
    nc = bacc.Bacc("TRN2", target_bir_lowering=False, debug=False,
                   num_devices=NCORES)
    xsT = nc.dram_tensor("xsT", [D, NS], F16, kind="ExternalInput").ap()
    W = {}
    for w in ("wqT", "wkT", "wvT", "w1T", "w2T", "w3T"):
        W[w] = nc.dram_tensor(w, [D, D], F16, kind="ExternalInput").ap()
    B = {}
    for b in ("bq", "b1", "b2", "b3"):
        B[b] = nc.dram_tensor(b, [D], F32, kind="ExternalInput").ap()
    fw = nc.dram_tensor("fw", [D], F16, kind="ExternalInput").ap()
    out = nc.dram_tensor("out", [1, NS], F32, kind="ExternalOutput").ap()

    with tile.TileContext(nc) as tc:
        with (
            tc.tile_pool(name="persist", bufs=1) as pers,
            tc.tile_pool(name="dram", bufs=1, space="DRAM") as dram,
        ):
            # ---- persistent SBUF tiles ----
            qt = pers.tile([P, DT, NS], F16, tag="qt")          # Q^T
            wT = {w: pers.tile([P, DT, D], F16, tag=f"{w}", name=f"{w}")
                  for w in ("w1T", "w2T", "w3T")}
            bsb = {b: pers.tile([P, DT], F32, tag=f"{b}sb", name=f"{b}sb")
                   for b in B}
            fwh = pers.tile([P, DT], F16, tag="fwh")
            ones_h = pers.tile([P, 1], F16, tag="ones")
            ones_row = pers.tile([1, P], F16, tag="ones_row")
            rs = pers.tile([1, NS], F32, tag="rs")              # softmax denom
            rs_h = pers.tile([1, NS], F16, tag="rs_h")

            # ---- DRAM scratch: per-chunk flat [kt][K-unit | V-unit] buffers
            csz = [2 * USZ * len(k) for k in CHUNK_KTS]
            kv_d = [dram.tile([csz[c]], F16, name=f"kv_d{c}")
                    for c in range(3)]
            kvag = [dram.tile([NCORES * csz[c]], F16, name=f"kvag{c}",
                              addr_space="Shared")
                    for c in range(3)]

            # ---- early pool: dies after projections ----
            early = tc.alloc_tile_pool(name="early", bufs=1)
            xsb = early.tile([P, DT, NS], F16, tag="xsb")
            wesb = {w: early.tile([P, DT, D], F16, tag=f"{w}", name=f"{w}")
                    for w in ("wqT", "wkT", "wvT")}
            kts = early.tile([P, DT, NS], F16, tag="kts")       # K^T shard
            vs = early.tile([P, KTB, D], F16, tag="vs")         # V shard

            # x (first half first) and the K/V weights lead the DMA queue so
            # the first projection matmul can start as early as possible.
            nc.sync.dma_start(xsb[:, :, 0:512],
                              xsT[:, 0:512].rearrange("(t p) n -> p t n", p=P))
            nc.sync.dma_start(wesb["wkT"][:],
                              W["wkT"].rearrange("(t p) m -> p t m", p=P))
            nc.sync.dma_start(wesb["wvT"][:],
                              W["wvT"].rearrange("(t p) m -> p t m", p=P))
            nc.sync.dma_start(xsb[:, :, 512:1024],
                              xsT[:, 512:1024].rearrange("(t p) n -> p t n",
                                                         p=P))
            nc.sync.dma_start(wesb["wqT"][:],
                              W["wqT"].rearrange("(t p) m -> p t m", p=P))
            for b in B:
                nc.sync.dma_start(bsb[b][:], B[b].rearrange("(t p) -> p t", p=P))
            nc.sync.dma_start(fwh[:], fw.rearrange("(t p) -> p t", p=P))
            nc.gpsimd.memset(ones_h[:], 1.0)
            nc.gpsimd.memset(ones_row[:], 1.0)

            def kproj(ppj, h):
                for dt in range(DT):
                    ps = ppj.tile([P, 512], F32, tag="ppj")
                    for et in range(DT):
                        nc.tensor.matmul(
                            ps[:],
                            wesb["wkT"][:, et, dt * P:(dt + 1) * P],
                            xsb[:, et, h * 512:(h + 1) * 512],
                            start=(et == 0), stop=(et == DT - 1))
                    nc.scalar.activation(
                        kts[:, dt, h * 512:(h + 1) * 512], ps[:], AF.Copy)

            def vproj(ppj, kt):
                for dh in range(2):
                    ps = ppj.tile([P, 512], F32, tag="ppj")
                    for et in range(DT):
                        nc.tensor.matmul(
                            ps[:],
                            xsb[:, et, kt * P:(kt + 1) * P],
                            wesb["wvT"][:, et, dh * 512:(dh + 1) * 512],
                            start=(et == 0), stop=(et == DT - 1))
                    nc.scalar.activation(
                        vs[:, kt, dh * 512:(dh + 1) * 512], ps[:], AF.Copy)

            def ship(c):
                for u, g in enumerate(CHUNK_KTS[c]):
                    off = 2 * USZ * u
                    nc.sync.dma_start(
                        kv_d[c][off:off + USZ].rearrange(
                            "(t p k) -> p t k", p=P, k=P),
                        kts[:, :, g * P:(g + 1) * P])
                    nc.sync.dma_start(
                        kv_d[c][off + USZ:off + 2 * USZ].rearrange(
                            "(p d) -> p d", d=D),
                        vs[:, g, :])
                nc.gpsimd.collective_compute(
                    "AllGather", ALU.bypass,
                    replica_groups=[list(range(NCORES))],
                    ins=[kv_d[c].opt()], outs=[kvag[c].opt()])

            with tc.tile_pool(name="ppj", bufs=4, space="PSUM") as ppj:
                # K^T = Wk @ xs^T (no bias: bk cancels in softmax);
                # V = xs @ Wv.T (bias folded into b1'). Ship chunk 0 (one kt)
                # as early as possible, then the rest, then project Q under
                # the first AllGather.
                kproj(ppj, 0)
                vproj(ppj, 0)
                ship(0)
                kproj(ppj, 1)
                for kt in (1, 2):
                    vproj(ppj, kt)
                ship(1)
                for kt in (3, 4, 5, 6, 7):
                    vproj(ppj, kt)
                ship(2)
                for dt in range(DT):
                    for h in range(2):
                        ps = ppj.tile([P, 512], F32, tag="ppj")
                        for et in range(DT):
                            nc.tensor.matmul(
                                ps[:],
                                wesb["wqT"][:, et, dt * P:(dt + 1) * P],
                                xsb[:, et, h * 512:(h + 1) * 512],
                                start=(et == 0), stop=(et == DT - 1))
                        nc.vector.tensor_tensor(
                            qt[:, dt, h * 512:(h + 1) * 512], ps[:],
                            bsb["bq"][:, dt:dt + 1].to_broadcast([P, 512]),
                            ALU.add)
            early.release()

            # ---- attention: 64 (block, kt) units in 8 groups of 8 ----
            units = []
            for c, kgl in enumerate(CHUNK_KTS):
                for kb in range(KB):
                    for u, _ in enumerate(kgl):
                        units.append((c, kb, u))
            groups = [units[i:i + G] for i in range(0, len(units), G)]

            pacc = tc.alloc_tile_pool(name="pacc", bufs=1)
            attacc = pacc.tile([P, DT, NS], F32, tag="attacc")
            acts = tc.alloc_tile_pool(name="acts", bufs=2)
            with (
                tc.tile_pool(name="kv", bufs=12) as kv,
                tc.tile_pool(name="ex", bufs=2 * G) as exp_pool,
                tc.tile_pool(name="psc", bufs=2, space="PSUM") as psc,
                tc.tile_pool(name="pat", bufs=5, space="PSUM") as pat,
                tc.tile_pool(name="prs", bufs=1, space="PSUM") as prs,
            ):
                recips = []
                for gi, group in enumerate(groups):
                    first_g = gi == 0
                    last_g = gi == len(groups) - 1
                    tiles = []
                    for (c, kb, u) in group:
                        off = kb * csz[c] + 2 * USZ * u
                        ktb = kv.tile([P, DT, P], F16, tag="ktb")
                        vb = kv.tile([P, D], F16, tag="vb")
                        nc.sync.dma_start(
                            ktb[:],
                            kvag[c][off:off + USZ].rearrange(
                                "(t p k) -> p t k", p=P, k=P))
                        nc.sync.dma_start(
                            vb[:],
                            kvag[c][off + USZ:off + 2 * USZ].rearrange(
                                "(p d) -> p d", d=D))
                        tiles.append((ktb, vb))
                    all_exs = []
                    for qp in range(2):
                        qpsl = slice(qp * 512, (qp + 1) * 512)
                        rs_ps = prs.tile([1, 512], F32, tag="prs")
                        exs = []
                        for ui, (ktb, vb) in enumerate(tiles):
                            sc = psc.tile([P, 512], F32, tag="psc")
                            for dt in range(DT):
                                nc.tensor.matmul(
                                    sc[:], ktb[:, dt, :], qt[:, dt, qpsl],
                                    start=(dt == 0), stop=(dt == DT - 1))
                            ex = exp_pool.tile([P, 512], F16, tag="ex")
                            nc.scalar.activation(ex[:], sc[:], AF.Exp,
                                                 scale=float(SCALE))
                            nc.tensor.matmul(rs_ps[:], ones_h[:], ex[:],
                                             start=(ui == 0),
                                             stop=(ui == G - 1),
                                             skip_group_check=True)
                            exs.append(ex)
                        if first_g:
                            nc.vector.tensor_copy(rs[0:1, qpsl], rs_ps[:])
                        else:
                            nc.vector.tensor_tensor(
                                rs[0:1, qpsl], rs_ps[:], rs[0:1, qpsl],
                                ALU.add)
                        all_exs.append(exs)
                    if last_g:
                        # rs is now complete: broadcast 1/rs while the PE is
                        # busy with this group's A@V chains below.
                        nc.vector.tensor_copy(rs_h[:], rs[:])
                        rbs = []
                        for h in range(2):
                            rb = pat.tile([P, 512], F32, tag="pat")
                            nc.tensor.matmul(
                                rb[:], ones_row[:],
                                rs_h[0:1, h * 512:(h + 1) * 512])
                            rbs.append(rb)
                        for h in range(2):
                            recip = acts.tile([P, 512], F32, tag="recip",
                                              name=f"recip{h}")
                            scr = acts.tile([P, 512], F32, tag="rscratch",
                                            name=f"rscratch{h}")
                            nc.vector.reciprocal_approx_accurate(
                                recip[:], rbs[h][:], scr[:])
                            recips.append(recip)
                    for qp in range(2):
                        qpsl = slice(qp * 512, (qp + 1) * 512)
                        exs = all_exs[qp]
                        for dh in range(2):
                            att_ps = [pat.tile([P, 512], F32, tag="pat",
                                               name=f"att_ps{_j}")
                                      for _j in range(4)]
                            for j in range(4):
                                dt = dh * 4 + j
                                for ui, (ktb, vb) in enumerate(tiles):
                                    nc.tensor.matmul(
                                        att_ps[j][:],
                                        vb[:, dt * P:(dt + 1) * P],
                                        exs[ui][:],
                                        start=(ui == 0), stop=(ui == G - 1),
                                        skip_group_check=True)
                            for j in range(4):
                                dsl = (slice(None), dh * 4 + j, qpsl)
                                if first_g:
                                    nc.vector.tensor_copy(attacc[dsl],
                                                          att_ps[j][:])
                                else:
                                    nc.vector.tensor_tensor(
                                        attacc[dsl], att_ps[j][:],
                                        attacc[dsl], ALU.add)

                # DMA the MLP weights (the queue drains these long before
                # the MLP starts).
                for w in ("w1T", "w2T", "w3T"):
                    nc.sync.dma_start(
                        wT[w][:], W[w].rearrange("(t p) m -> p t m", p=P))

            # ---- normalize + MLP + final, pipelined by column half ----
            with tc.tile_pool(name="pml", bufs=4, space="PSUM") as pml:
                out_sb = acts.tile([1, NS], F32, tag="out_sb")
                for h in range(2):
                    qsl = slice(h * 512, (h + 1) * 512)
                    attn_h = acts.tile([P, DT, 512], F16, tag="y",
                                       name=f"attn_h{h}")
                    for dt in range(DT):
                        nc.vector.tensor_tensor(
                            attn_h[:, dt, :], attacc[:, dt, qsl],
                            recips[h][:], ALU.mult)
                    cur = attn_h
                    for wname, bname in (("w1T", "b1"), ("w2T", "b2"),
                                         ("w3T", "b3")):
                        nxt = acts.tile([P, DT, 512], F16, tag="y",
                                        name=f"{wname}y{h}")
                        for ft in range(DT):
                            ps = pml.tile([P, 512], F32, tag="pml")
                            for dt in range(DT):
                                nc.tensor.matmul(
                                    ps[:],
                                    wT[wname][:, dt, ft * P:(ft + 1) * P],
                                    cur[:, dt, :],
                                    start=(dt == 0), stop=(dt == DT - 1))
                            nc.scalar.activation(
                                nxt[:, ft, :], ps[:],
                                AF.Relu, bias=bsb[bname][:, ft:ft + 1])
                        cur = nxt
                    ps = pml.tile([1, 512], F32, tag="pfin")
                    for ft in range(DT):
                        nc.tensor.matmul(
                            ps[:], fwh[:, ft:ft + 1], cur[:, ft, :],
                            start=(ft == 0), stop=(ft == DT - 1))
                    nc.vector.tensor_copy(out_sb[0:1, qsl], ps[:])
                nc.sync.dma_start(out[:], out_sb[:])
            acts.release()
            pacc.release()

    nc.compile()
    return nc


def _get_nc():
    if "nc" not in _CACHE:
        _CACHE["nc"] = _build()
    return _CACHE["nc"]


def make_in_maps(inputs):
    """Host-side sharding/layout: transpose + fp16-cast the weights and the
    x shards, fold bv into b1."""
    f32 = np.float32
    x = np.asarray(inputs["x"], dtype=f32)
    shared = {}
    for dev, ref in (("wqT", "Wq"), ("wkT", "Wk"), ("wvT", "Wv"),
                     ("w1T", "W1"), ("w2T", "W2"), ("w3T", "W3")):
        shared[dev] = np.ascontiguousarray(
            np.asarray(inputs[ref], dtype=f32).T.astype(np.float16))
    b1p = (np.asarray(inputs["b1"], dtype=f32)
           + np.asarray(inputs["W1"], dtype=f32)
           @ np.asarray(inputs["bv"], dtype=f32)).astype(f32)
    shared["bq"] = np.ascontiguousarray(np.asarray(inputs["bq"], dtype=f32))
    shared["b1"] = np.ascontiguousarray(b1p)
    shared["b2"] = np.ascontiguousarray(np.asarray(inputs["b2"], dtype=f32))
    shared["b3"] = np.ascontiguousarray(np.asarray(inputs["b3"], dtype=f32))
    shared["fw"] = np.ascontiguousarray(
        np.asarray(inputs["final_weight"], dtype=f32).reshape(D)
        .astype(np.float16))
    in_maps = []
    for c in range(NCORES):
        m = dict(shared)
        m["xsT"] = np.ascontiguousarray(
            x[c * NS:(c + 1) * NS, :].T.astype(np.float16))
        in_maps.append(m)
    return in_maps


def kernel(**inputs):
    nc = _get_nc()
    res = bass_utils.run_bass_kernel_spmd(
        nc, make_in_maps(inputs), core_ids=list(range(NCORES)))
    return np.concatenate(
        [res.results[c]["out"].reshape(NS) for c in range(NCORES)])


# revision 18
# speedup vs baseline: 1.3157x; 1.0284x over previous
"""Trainium2 Bass kernel for DeepSelfAttention (N=8192, D=1024) on 8 NeuronCores.

Strategy (row-parallel attention):
  - Shard the N=8192 rows of x across 8 cores (1024 rows each); replicate
    weights.  All weight matrices and the x shards are pre-transposed and cast
    to fp16 on the HOST, so the device does zero layout work: every matmul
    operand is DMA'd directly into its contraction-major SBUF layout.
  - Bias algebra (host-folded):
      * bk drops out of softmax entirely (it shifts every score of a row by a
        q-dependent constant).
      * bv is folded into the first MLP bias: b1' = b1 + W1 @ bv (softmax rows
        sum to 1).
    Only bq survives on device (added to Q after the projection).
  - Each core computes K^T and V for its row shard and ships them in three
    fp16 chunks of [1, 2, 5] key-tiles; each chunk is AllGathered as soon as
    it is ready, so the first chunk's collective fully overlaps the remaining
    projections and each later chunk overlaps attention on earlier chunks.
  - Attention processes 128-key units in groups of 8 (1024 keys): scores^T
    tiles [k=128, q=512] accumulate over feature tiles in PSUM, exp on ScalarE
    (scale=1/32 fused; no max-subtraction needed, scores are small), softmax
    denominator via a ones-vector matmul, A@V accumulated across the whole
    group (8-matmul PSUM chains) and flushed to an SBUF fp32 accumulator.
  - Normalize via PE broadcast of 1/rowsum (emitted inside the last attention
    group so the DVE work hides under the last A@V chains), then the 3-layer
    MLP + final projection, feature-major, pipelined by column half.
All matmul operands are fp16 with fp32 PSUM accumulation; end-to-end max rel
err vs the fp32 reference is ~5e-4.
"""

import numpy as np

import concourse.mybir as mybir
import concourse.tile as tile
from concourse import bacc
from concourse import bass_utils

P = 128
D = 1024
N = 8192
NCORES = 8
NS = N // NCORES          # 1024 rows per core
DT = D // P               # 8 feature tiles
KB = 8                    # k blocks (one per source core)
KTB = NS // P             # 8 k tiles per block
CHUNK_KTS = [[0], [1, 2], [3, 4, 5, 6, 7]]   # kt split per AllGather chunk
USZ = P * D               # elements per K or V unit (128 keys x 1024 feat)
G = 8                     # units per attention group (PSUM chain length)
F16 = mybir.dt.float16
F32 = mybir.dt.float32
AF = mybir.ActivationFunctionType
ALU = mybir.AluOpType

SCALE = 1.0 / np.sqrt(np.float32(D)).astype(np.float32)  # 0.03125

_CACHE = {}


def _build():
    nc = bacc.Bacc("TRN2", target_bir_lowering=False, debug=False,
                   num_devices=NCORES)
    xsT = nc.dram_tensor("xsT", [D, NS], F16, kind="ExternalInput").ap()
    W = {}
    for w in ("wqT", "wkT", "wvT", "w1T", "w2T", "w3T"):
        W[w] = nc.dram_tensor(w, [D, D], F16, kind="ExternalInput").ap()
    B = {}
    for b in ("bq", "b1", "b2", "b3"):
        B[b] = nc.dram_tensor(b, [D], F32, kind="ExternalInput").ap()
    fw = nc.dram_tensor("fw", [D], F16, kind="ExternalInput").ap()
    out = nc.dram_tensor("out", [1, NS], F32, kind="ExternalOutput").ap()

    with tile.TileContext(nc) as tc:
        with (
            tc.tile_pool(name="persist", bufs=1) as pers,
            tc.tile_pool(name="dram", bufs=1, space="DRAM") as dram,
        ):
            # ---- persistent SBUF tiles ----
            qt = pers.tile([P, DT, NS], F16, tag="qt")          # Q^T
            wT = {w: pers.tile([P, DT, D], F16, tag=f"{w}", name=f"{w}")
                  for w in ("w1T", "w2T", "w3T")}
            bsb = {b: pers.tile([P, DT], F32, tag=f"{b}sb", name=f"{b}sb")
                   for b in B}
            fwh = pers.tile([P, DT], F16, tag="fwh")
            ones_h = pers.tile([P, 1], F16, tag="ones")
            ones_row = pers.tile([1, P], F16, tag="ones_row")
            rs = pers.tile([1, NS], F32, tag="rs")              # softmax denom
            rs_h = pers.tile([1, NS], F16, tag="rs_h")

            # ---- DRAM scratch: per-chunk flat [kt][K-unit | V-unit] buffers
            csz = [2 * USZ * len(k) for k in CHUNK_KTS]
            kv_d = [dram.tile([csz[c]], F16, name=f"kv_d{c}")
                    for c in range(3)]
            kvag = [dram.tile([NCORES * csz[c]], F16, name=f"kvag{c}",
                              addr_space="Shared")
                    for c in range(3)]

            # ---- early pool: dies after projections ----
            early = tc.alloc_tile_pool(name="early", bufs=1)
            xsb = early.tile([P, DT, NS], F16, tag="xsb")
            wesb = {w: early.tile([P, DT, D], F16, tag=f"{w}", name=f"{w}")
                    for w in ("wqT", "wkT", "wvT")}
            kts = early.tile([P, DT, NS], F16, tag="kts")       # K^T shard
            vs = early.tile([P, KTB, D], F16, tag="vs")         # V shard

            # x (first half first) and the K/V weights lead the DMA queue so
            # the first projection matmul can start as early as possible.
            nc.sync.dma_start(xsb[:, :, 0:512],
                              xsT[:, 0:512].rearrange("(t p) n -> p t n", p=P))
            nc.sync.dma_start(wesb["wkT"][:],
                              W["wkT"].rearrange("(t p) m -> p t m", p=P))
            nc.sync.dma_start(wesb["wvT"][:],
                              W["wvT"].rearrange("(t p) m -> p t m", p=P))
            nc.sync.dma_start(xsb[:, :, 512:1024],
                              xsT[:, 512:1024].rearrange("(t p) n -> p t n",
                                                         p=P))
            nc.sync.dma_start(wesb["wqT"][:],
                              W["wqT"].rearrange("(t p) m -> p t m", p=P))
            for b in B:
                nc.sync.dma_start(bsb[b][:], B[b].rearrange("(t p) -> p t", p=P))
            nc.sync.dma_start(fwh[:], fw.rearrange("(t p) -> p t", p=P))
            nc.gpsimd.memset(ones_h[:], 1.0)
            nc.gpsimd.memset(ones_row[:], 1.0)

            def kproj(ppj, h):
                for dt in range(DT):
                    ps = ppj.tile([P, 512], F32, tag="ppj")
                    for et in range(DT):
                        nc.tensor.matmul(
                            ps[:],
                            wesb["wkT"][:, et, dt * P:(dt + 1) * P],
                            xsb[:, et, h * 512:(h + 1) * 512],
                            start=(et == 0), stop=(et == DT - 1))
                    nc.scalar.activation(
                        kts[:, dt, h * 512:(h + 1) * 512], ps[:], AF.Copy)

            def vproj(ppj, kt):
                for dh in range(2):
                    ps = ppj.tile([P, 512], F32, tag="ppj")
                    for et in range(DT):
                        nc.tensor.matmul(
                            ps[:],
                            xsb[:, et, kt * P:(kt + 1) * P],
                            wesb["wvT"][:, et, dh * 512:(dh + 1) * 512],
                            start=(et == 0), stop=(et == DT - 1))
                    nc.scalar.activation(
                        vs[:, kt, dh * 512:(dh + 1) * 512], ps[:], AF.Copy)

            def ship(c):
                for u, g in enumerate(CHUNK_KTS[c]):
                    off = 2 * USZ * u
                    nc.sync.dma_start(
                        kv_d[c][off:off + USZ].rearrange(
                            "(p t k) -> p t k", p=P, k=P),
                        kts[:, :, g * P:(g + 1) * P])
                    nc.sync.dma_start(
                        kv_d[c][off + USZ:off + 2 * USZ].rearrange(
                            "(p d) -> p d", d=D),
                        vs[:, g, :])
                nc.gpsimd.collective_compute(
                    "AllGather", ALU.bypass,
                    replica_groups=[list(range(NCORES))],
                    ins=[kv_d[c].opt()], outs=[kvag[c].opt()])

            with tc.tile_pool(name="ppj", bufs=4, space="PSUM") as ppj:
                # K^T = Wk @ xs^T (no bias: bk cancels in softmax);
                # V = xs @ Wv.T (bias folded into b1'). Ship chunk 0 (one kt)
                # as early as possible, then the rest, then project Q under
                # the first AllGather.
                kproj(ppj, 0)
                vproj(ppj, 0)
                ship(0)
                kproj(ppj, 1)
                for kt in (1, 2):
                    vproj(ppj, kt)
                ship(1)
                for kt in (3, 4, 5, 6, 7):
                    vproj(ppj, kt)
                ship(2)
                for dt in range(DT):
                    for h in range(2):
                        ps = ppj.tile([P, 512], F32, tag="ppj")
                        for et in range(DT):
                            nc.tensor.matmul(
                                ps[:],
                                wesb["wqT"][:, et, dt * P:(dt + 1) * P],
                                xsb[:, et, h * 512:(h + 1) * 512],
                                start=(et == 0), stop=(et == DT - 1))
                        nc.vector.tensor_tensor(
                            qt[:, dt, h * 512:(h + 1) * 512], ps[:],
                            bsb["bq"][:, dt:dt + 1].to_broadcast([P, 512]),
                            ALU.add)
            early.release()

            # ---- attention: 64 (block, kt) units in 8 groups of 8 ----
            units = []
            for c, kgl in enumerate(CHUNK_KTS):
                for kb in range(KB):
                    for u, _ in enumerate(kgl):
                        units.append((c, kb, u))
            groups = [units[i:i + G] for i in range(0, len(units), G)]

            pacc = tc.alloc_tile_pool(name="pacc", bufs=1)
            attacc = pacc.tile([P, DT, NS], F32, tag="attacc")
            acts = tc.alloc_tile_pool(name="acts", bufs=3)
            att0 = tc.alloc_tile_pool(name="att0", bufs=1)
            with (
                tc.tile_pool(name="kv", bufs=10) as kv,
                tc.tile_pool(name="ex", bufs=2 * G) as exp_pool,
                tc.tile_pool(name="psc", bufs=2, space="PSUM") as psc,
                tc.tile_pool(name="pat", bufs=5, space="PSUM") as pat,
                tc.tile_pool(name="prs", bufs=1, space="PSUM") as prs,
            ):
                recips = []
                for gi, group in enumerate(groups):
                    first_g = gi == 0
                    last_g = gi == len(groups) - 1
                    tiles = []
                    for (c, kb, u) in group:
                        off = kb * csz[c] + 2 * USZ * u
                        ktb = kv.tile([P, DT, P], F16, tag="ktb")
                        vb = kv.tile([P, D], F16, tag="vb")
                        nc.sync.dma_start(
                            ktb[:],
                            kvag[c][off:off + USZ].rearrange(
                                "(p t k) -> p t k", p=P, k=P))
                        nc.sync.dma_start(
                            vb[:],
                            kvag[c][off + USZ:off + 2 * USZ].rearrange(
                                "(p d) -> p d", d=D))
                        tiles.append((ktb, vb))
                    all_exs = []
                    for qp in range(2):
                        qpsl = slice(qp * 512, (qp + 1) * 512)
                        rs_ps = prs.tile([1, 512], F32, tag="prs")
                        exs = []
                        for ui, (ktb, vb) in enumerate(tiles):
                            sc = psc.tile([P, 512], F32, tag="psc")
                            for dt in range(DT):
                                nc.tensor.matmul(
                                    sc[:], ktb[:, dt, :], qt[:, dt, qpsl],
                                    start=(dt == 0), stop=(dt == DT - 1))
                            ex = exp_pool.tile([P, 512], F16, tag="ex")
                            nc.scalar.activation(ex[:], sc[:], AF.Exp,
                                                 scale=float(SCALE))
                            exs.append(ex)
                        # denominator matmuls after all scores chains so they
                        # never wait on the ScalarE exp of their operand
                        for ui in range(G):
                            nc.tensor.matmul(rs_ps[:], ones_h[:], exs[ui][:],
                                             start=(ui == 0),
                                             stop=(ui == G - 1),
                                             skip_group_check=True)
                        if first_g:
                            nc.vector.tensor_copy(rs[0:1, qpsl], rs_ps[:])
                        else:
                            nc.vector.tensor_tensor(
                                rs[0:1, qpsl], rs_ps[:], rs[0:1, qpsl],
                                ALU.add)
                        all_exs.append(exs)
                    if last_g:
                        # rs is now complete: broadcast 1/rs while the PE is
                        # busy with this group's A@V chains below.
                        nc.vector.tensor_copy(rs_h[:], rs[:])
                        rbs = []
                        for h in range(2):
                            rb = pat.tile([P, 512], F32, tag="pat")
                            nc.tensor.matmul(
                                rb[:], ones_row[:],
                                rs_h[0:1, h * 512:(h + 1) * 512])
                            rbs.append(rb)
                        for h in range(2):
                            recip = acts.tile([P, 512], F32, tag="recip",
                                              name=f"recip{h}")
                            scr = acts.tile([P, 512], F32, tag="rscratch",
                                            name=f"rscratch{h}")
                            nc.vector.reciprocal_approx_accurate(
                                recip[:], rbs[h][:], scr[:])
                            recips.append(recip)
                    for qp in range(2):
                        qpsl = slice(qp * 512, (qp + 1) * 512)
                        exs = all_exs[qp]
                        for dh in range(2):
                            att_ps = [pat.tile([P, 512], F32, tag="pat",
                                               name=f"att_ps{_j}")
                                      for _j in range(4)]
                            for j in range(4):
                                dt = dh * 4 + j
                                for ui, (ktb, vb) in enumerate(tiles):
                                    nc.tensor.matmul(
                                        att_ps[j][:],
                                        vb[:, dt * P:(dt + 1) * P],
                                        exs[ui][:],
                                        start=(ui == 0), stop=(ui == G - 1),
                                        skip_group_check=True)
                            for j in range(4):
                                dsl = (slice(None), dh * 4 + j, qpsl)
                                if first_g:
                                    nc.vector.tensor_copy(attacc[dsl],
                                                          att_ps[j][:])
                                else:
                                    nc.vector.tensor_tensor(
                                        attacc[dsl], att_ps[j][:],
                                        attacc[dsl], ALU.add)
                        if last_g and qp == 0:
                            # column half 0 is now complete: normalize it on
                            # the DVE while the PE runs the qp=1 A@V chains,
                            # so the MLP starts with zero stall.
                            attn_pre = att0.tile([P, DT, 512], F16, tag="y0",
                                                 name="attn_h0")
                            for dt in range(DT):
                                nc.vector.tensor_tensor(
                                    attn_pre[:, dt, :], attacc[:, dt, qpsl],
                                    recips[0][:], ALU.mult)

                # DMA the MLP weights (the queue drains these long before
                # the MLP starts).
                for w in ("w1T", "w2T", "w3T"):
                    nc.sync.dma_start(
                        wT[w][:], W[w].rearrange("(t p) m -> p t m", p=P))

            # ---- normalize + MLP + final, pipelined by column half ----
            with (
                tc.tile_pool(name="pml", bufs=4, space="PSUM") as pml,
                tc.tile_pool(name="outp", bufs=1) as outp,
            ):
                out_sb = outp.tile([1, NS], F32, tag="out_sb")
                for h in range(2):
                    qsl = slice(h * 512, (h + 1) * 512)
                    if h == 0:
                        attn_h = attn_pre
                    else:
                        attn_h = acts.tile([P, DT, 512], F16, tag="y",
                                           name=f"attn_h{h}")
                        for dt in range(DT):
                            nc.vector.tensor_tensor(
                                attn_h[:, dt, :], attacc[:, dt, qsl],
                                recips[h][:], ALU.mult)
                    cur = attn_h
                    for wname, bname in (("w1T", "b1"), ("w2T", "b2"),
                                         ("w3T", "b3")):
                        nxt = acts.tile([P, DT, 512], F16, tag="y",
                                        name=f"{wname}y{h}")
                        for ft in range(DT):
                            ps = pml.tile([P, 512], F32, tag="pml")
                            for dt in range(DT):
                                nc.tensor.matmul(
                                    ps[:],
                                    wT[wname][:, dt, ft * P:(ft + 1) * P],
                                    cur[:, dt, :],
                                    start=(dt == 0), stop=(dt == DT - 1))
                            nc.scalar.activation(
                                nxt[:, ft, :], ps[:],
                                AF.Relu, bias=bsb[bname][:, ft:ft + 1])
                        cur = nxt
                    ps = pml.tile([1, 512], F32, tag="pfin")
                    for ft in range(DT):
                        nc.tensor.matmul(
                            ps[:], fwh[:, ft:ft + 1], cur[:, ft, :],
                            start=(ft == 0), stop=(ft == DT - 1))
                    nc.vector.tensor_copy(out_sb[0:1, qsl], ps[:])
                nc.sync.dma_start(out[:], out_sb[:])
            att0.release()
            acts.release()
            pacc.release()

    nc.compile()
    return nc


def _get_nc():
    if "nc" not in _CACHE:
        _CACHE["nc"] = _build()
    return _CACHE["nc"]


def make_in_maps(inputs):
    """Host-side sharding/layout: transpose + fp16-cast the weights and the
    x shards, fold bv into b1."""
    f32 = np.float32
    x = np.asarray(inputs["x"], dtype=f32)
    shared = {}
    for dev, ref in (("wqT", "Wq"), ("wkT", "Wk"), ("wvT", "Wv"),
                     ("w1T", "W1"), ("w2T", "W2"), ("w3T", "W3")):
        shared[dev] = np.ascontiguousarray(
            np.asarray(inputs[ref], dtype=f32).T.astype(np.float16))
    b1p = (np.asarray(inputs["b1"], dtype=f32)
           + np.asarray(inputs["W1"], dtype=f32)
           @ np.asarray(inputs["bv"], dtype=f32)).astype(f32)
    shared["bq"] = np.ascontiguousarray(np.asarray(inputs["bq"], dtype=f32))
    shared["b1"] = np.ascontiguousarray(b1p)
    shared["b2"] = np.ascontiguousarray(np.asarray(inputs["b2"], dtype=f32))
    shared["b3"] = np.ascontiguousarray(np.asarray(inputs["b3"], dtype=f32))
    shared["fw"] = np.ascontiguousarray(
        np.asarray(inputs["final_weight"], dtype=f32).reshape(D)
        .astype(np.float16))
    in_maps = []
    for c in range(NCORES):
        m = dict(shared)
        m["xsT"] = np.ascontiguousarray(
            x[c * NS:(c + 1) * NS, :].T.astype(np.float16))
        in_maps.append(m)
    return in_maps


def kernel(**inputs):
    nc = _get_nc()
    res = bass_utils.run_bass_kernel_spmd(
        nc, make_in_maps(inputs), core_ids=list(range(NCORES)))
    return np.concatenate(
        [res.results[c]["out"].reshape(NS) for c in range(NCORES)])
